# revision 39
# baseline (speedup 1.0000x reference)
"""2D Gaussian Splatting on 8 Trainium2 NeuronCores — layout-B cumprod design.

Pixels live on partitions: each pixel-tile is 16x8 = 128 px. Per tile, the
culled gaussian list (2.25-sigma bbox, global index order) occupies a run of
free-dim columns: [spacer, g0..g_{L-1}, pads]. The 512 tiles are globally
sorted by gaussian count and dealt round-robin to the 8 cores, so every core
holds 64 tiles in 8 buckets of 8 segments; bucket heights L_j are global
maxima, making the packed geometry identical across cores (one SPMD program).

Raw Bass (no TileContext): the Tile teardown serially resets ~51 semaphores
per queue (~9us tail); hand-rolled sync with 7 semaphores avoids it.

  zb  = basisT @ coefs (+ logopac bias rows)      fp32r matmul K=9
  zbc = basisT @ coefs (+ (logopac+ln c) rows)    second matmul, shared rhs
        (spacer/pad cols: zb bias 0 -> alpha=1, om=0; zbc bias -60 -> AC=0)
  512-col chunks into two fixed 4-bank PSUM tiles (no reuse, no WAR sync).
  alpha = Exp(zb) [ACT f32]     AC = Exp(zbc) [ACT bf16]
  om = 1 - alpha  [GP tensor_scalar f32]
  s  = segmented cumprod: scan state' = max(om*state, d1p) [DVE, bf16 out]
       d1p built on-device: memset 0 + strided memset 1.0 at spacer cols
  wc = AC * s_shifted_one_col  [DVE bf16]  (0 at spacers/pads since AC=0)
  img column = per-bucket 3D tensor_reduce over segments  [DVE]
  out [128, 64] f32 -> host places each column as a 16x8 pixel block.
"""

import math
from contextlib import ExitStack

import numpy as np

W = 256
H = 256
TW = 16            # pixel tile width
TH = 8             # pixel tile height
NTX = W // TW      # 16
NTY = H // TH      # 32
NTILES = NTX * NTY # 512
N_CORES = 8
NT_CORE = NTILES // N_CORES   # 64 tiles per core
NSEG = 8                      # segments per bucket
NBUCK = NT_CORE // NSEG       # 8 buckets
SIGMA_K = 2.25
KQ = 9             # 5 coef rows + bias hi/lo + colorbias hi/lo
CW = 512           # matmul chunk width (one PSUM bank of f32)
NEG = -60.0        # exp(NEG) == 0 for spacer/pad color bias


def _round_fp32r(a):
    b = np.asarray(a, np.float32).view(np.uint32).astype(np.uint64)
    r = (b + 0x7FF + ((b >> 12) & 1)) & 0xFFFFF000
    return r.astype(np.uint32).view(np.float32)


def _split_fp32r(a):
    a = np.asarray(a, np.float32)
    hi = _round_fp32r(a)
    lo = _round_fp32r(a - hi)
    return hi, lo


def _build_nc(gcap, lbs):
    """lbs: list of NBUCK bucket heights L_j (segment width is L_j + 1)."""
    import concourse.bacc as bacc
    import concourse.mybir as mybir

    f32 = mybir.dt.float32
    f32r = mybir.dt.float32r
    bf16 = mybir.dt.bfloat16
    AF = mybir.ActivationFunctionType
    OP = mybir.AluOpType

    assert gcap <= 4 * CW, f"gcap {gcap} exceeds fixed PSUM layout"
    chunks = []
    c0 = 0
    while c0 < gcap:
        cw = min(CW, gcap - c0)
        chunks.append((c0, cw))
        c0 += cw

    nc = bacc.Bacc("TRN2", target_bir_lowering=False, debug=False)
    inp_d = nc.declare_dram_parameter("inp", [KQ, 256 + gcap], f32r, isOutput=False)
    out_d = nc.declare_dram_parameter("out", [128, NT_CORE], f32, isOutput=True)

    es = ExitStack()
    nc._keepalive_es = es   # tensors must stay allocated through compile
    inp_t = es.enter_context(nc.sbuf_tensor("inp_t", [KQ, 256 + gcap], f32r))
    d1p_t = es.enter_context(nc.sbuf_tensor("d1p_t", [128, gcap], bf16))
    abuf = es.enter_context(nc.sbuf_tensor("abuf", [128, gcap], f32))
    acbuf = es.enter_context(nc.sbuf_tensor("acbuf", [128, gcap], bf16))
    ombuf = es.enter_context(nc.sbuf_tensor("ombuf", [128, gcap], f32))
    sbufS = es.enter_context(nc.sbuf_tensor("sbufS", [128, gcap + 1], bf16))
    wcbuf = es.enter_context(nc.sbuf_tensor("wcbuf", [128, gcap], bf16))
    outsb = es.enter_context(nc.sbuf_tensor("outsb", [128, NT_CORE], f32))
    dummy = es.enter_context(nc.sbuf_tensor("gs_dummy", [1, 2], f32))
    psZ = es.enter_context(nc.psum_tensor("psZ", [128, 4 * CW], f32))
    psC = es.enter_context(nc.psum_tensor("psC", [128, 4 * CW], f32))

    s_g0 = nc.alloc_semaphore("s_g0")
    s_ginit = nc.alloc_semaphore("s_ginit")
    s_inp0 = nc.alloc_semaphore("s_inp0")
    s_inp1 = nc.alloc_semaphore("s_inp1")
    s_mm = nc.alloc_semaphore("s_mm")
    s_a = nc.alloc_semaphore("s_a")
    s_ac = nc.alloc_semaphore("s_ac")
    s_om = nc.alloc_semaphore("s_om")
    s_red = nc.alloc_semaphore("s_red")
    s_out = nc.alloc_semaphore("s_out")
    s_d1z = nc.alloc_semaphore("s_d1z")

    lhsT = inp_t[:, 0:256]
    rhs = inp_t[:, 256 : 256 + gcap]

    with nc.Block() as blk:

        # elementwise/scan stages run on a coarser 3-chunk grid (the small
        # matmul remainder is merged into the last chunk) — one fewer
        # scan-chain hop; mm-completion waits are derived per ew-chunk.
        ew = [(0, 256), (256, 256), (CW, CW), (2 * CW, gcap - 2 * CW)]
        ew = [(a, w) for a, w in ew if w > 0]

        def mm_cnt(c0, cw):
            return 2 * ((c0 + cw - 1) // CW) + 1

        # bucket column extents for interleaved reduces
        bext = []
        off = 0
        for lb in lbs:
            seg = lb + 1
            bext.append((off, off + NSEG * seg))
            off += NSEG * seg

        @blk.gpsimd
        def _(gp):
            gp.memset(dummy[:], 0.0).then_inc(s_g0, 1)
            gp.memset(d1p_t[:], 0.0).then_inc(s_d1z, 1)
            gp.wait_ge(s_d1z, 1)
            off = 0
            for lb in lbs:
                seg = lb + 1
                ap3 = d1p_t[:, off : off + NSEG * seg].rearrange(
                    "p (s l) -> p s l", l=seg
                )
                gp.memset(ap3[:, :, 0:1], 1.0)
                off += NSEG * seg
            gp.memset(sbufS[:, 0:1], 0.0).then_inc(s_ginit, 1)
            for ci, (c0, cw) in enumerate(ew):
                sl = slice(c0, c0 + cw)
                gp.wait_ge(s_a, ci + 1)
                gp.tensor_scalar(
                    ombuf[:, sl], abuf[:, sl], -1.0, 1.0, OP.mult, OP.add
                ).then_inc(s_om, 1)

        @blk.sync
        def _(sync):
            sync.dma_start(
                inp_t[:, 0 : 256 + CW], inp_d[:, 0 : 256 + CW]
            ).then_inc(s_inp0, 16)
            sync.dma_start(
                inp_t[:, 256 + CW :], inp_d[:, 256 + CW :]
            ).then_inc(s_inp1, 16)
            sync.wait_ge(s_red, 1)
            sync.dma_start(out_d[:], outsb[:]).then_inc(s_out, 16)
            sync.wait_ge(s_out, 16)

        @blk.tensor
        def _(te):
            te.wait_ge(s_inp0, 16)
            for ci, (c0, cw) in enumerate(chunks):
                if ci == 1:
                    te.wait_ge(s_inp1, 16)
                te.matmul(
                    psZ[:, c0 : c0 + cw],
                    lhsT[:, 0:128],
                    rhs[:, c0 : c0 + cw],
                    start=True,
                    stop=True,
                ).then_inc(s_mm, 1)
                te.matmul(
                    psC[:, c0 : c0 + cw],
                    lhsT[:, 128:256],
                    rhs[:, c0 : c0 + cw],
                    start=True,
                    stop=True,
                ).then_inc(s_mm, 1)

        @blk.scalar
        def _(act):
            act.wait_ge(s_g0, 1)
            act.activation(dummy[0:1, 0:1], dummy[0:1, 1:2], AF.Exp, bias=0.0)
            order = [("a", 0), ("a", 1), ("c", 0), ("a", 2), ("c", 1),
                     ("a", 3), ("c", 2), ("c", 3)]
            for kind, ci in order:
                if ci >= len(ew):
                    continue
                c0, cw = ew[ci]
                sl = slice(c0, c0 + cw)
                if kind == "a":
                    act.wait_ge(s_mm, mm_cnt(c0, cw))
                    act.activation(
                        abuf[:, sl], psZ[:, sl], AF.Exp, bias=0.0
                    ).then_inc(s_a, 1)
                else:
                    act.wait_ge(s_mm, mm_cnt(c0, cw) + 1)
                    act.activation(
                        acbuf[:, sl], psC[:, sl], AF.Exp, bias=0.0
                    ).then_inc(s_ac, 1)

        @blk.vector
        def _(dve):
            done_buckets = 0
            inst = None
            for ci, (c0, cw) in enumerate(ew):
                sl = slice(c0, c0 + cw)
                dve.wait_ge(s_om, ci + 1)
                if ci == 0:
                    dve.wait_ge(s_ginit, 1)
                dve.tensor_tensor_scan(
                    sbufS[:, c0 + 1 : c0 + cw + 1],
                    ombuf[:, sl],
                    d1p_t[:, sl],
                    0.0 if ci == 0 else sbufS[:, c0 : c0 + 1],
                    OP.mult,
                    OP.max,
                )
                dve.wait_ge(s_ac, ci + 1)
                dve.tensor_mul(wcbuf[:, sl], acbuf[:, sl], sbufS[:, sl])
                # fire reduces whose bucket columns are fully covered
                covered = c0 + cw
                for j in range(done_buckets, NBUCK):
                    b0, b1 = bext[j]
                    if b1 > covered:
                        break
                    seg = lbs[j] + 1
                    ap3 = wcbuf[:, b0:b1].rearrange("p (s l) -> p s l", l=seg)
                    with nc.allow_low_precision("f32 out; gate trips on bf16 in"):
                        inst = dve.tensor_reduce(
                            outsb[:, j * NSEG : (j + 1) * NSEG],
                            ap3,
                            mybir.AxisListType.X,
                            OP.add,
                        )
                    done_buckets = j + 1
            assert done_buckets == NBUCK and inst is not None
            inst.then_inc(s_red, 1)

    nc.compile()
    return nc


_NC_CACHE = {}
LAST_RESULT = None


def _get_nc(gcap, lbs):
    key = (gcap, tuple(lbs))
    if key not in _NC_CACHE:
        _NC_CACHE[key] = _build_nc(gcap, lbs)
    return _NC_CACHE[key]


def _prep_inputs(means, quats, scales, rgbs, opacities):
    """Cull + pack per core. Returns (in_maps, tile_of, gcap, lbs)."""

    means = np.asarray(means, np.float64)
    quats = np.asarray(quats, np.float64)
    scales = np.asarray(scales, np.float64)
    rgbs = np.asarray(rgbs, np.float64)
    opacities = np.asarray(opacities, np.float64)

    c = np.cos(quats)
    s = np.sin(quats)
    sx2 = scales[:, 0] ** 2
    sy2 = scales[:, 1] ** 2
    a11 = c * c * sx2 + s * s * sy2
    a12 = c * s * (sx2 - sy2)
    a22 = s * s * sx2 + c * c * sy2
    det = a11 * a22 - a12 * a12
    ia = a22 / det
    ib = -a12 / det
    ic = a11 / det
    logopac = -np.logaddexp(0.0, -opacities)
    colors = 1.0 / (1.0 + np.exp(-rgbs[:, 0]))
    lnc = np.log(colors)
    rx = SIGMA_K * np.sqrt(a11)
    ry = SIGMA_K * np.sqrt(a22)
    x0g, x1g = means[:, 0] - rx, means[:, 0] + rx
    y0g, y1g = means[:, 1] - ry, means[:, 1] + ry

    tile_idx = []
    for t in range(NTILES):
        ty, tx = divmod(t, NTX)
        X0, X1 = tx * TW, (tx + 1) * TW
        Y0, Y1 = ty * TH, (ty + 1) * TH
        idx = np.nonzero(
            (x1g >= X0) & (x0g <= X1) & (y1g >= Y0) & (y0g <= Y1)
        )[0]
        tile_idx.append(idx)

    # global sort by count desc; rank r -> core r%8, position r//8
    order = sorted(range(NTILES), key=lambda t: -len(tile_idx[t]))
    lbs = [len(tile_idx[order[64 * j]]) for j in range(NBUCK)]
    gcap = sum(NSEG * (lb + 1) for lb in lbs)

    fx = (np.arange(128) % TW).astype(np.float64) - (TW - 1) / 2.0
    fy = (np.arange(128) // TW).astype(np.float64) - (TH - 1) / 2.0
    basis5 = _round_fp32r(np.stack([fx * fx, fx * fy, fy * fy, fx, fy]))
    lhsT = np.zeros((KQ, 256), np.float32)
    lhsT[0:5, 0:128] = basis5
    lhsT[5, 0:128] = 1.0
    lhsT[6, 0:128] = 1.0
    lhsT[0:5, 128:256] = basis5
    lhsT[7, 128:256] = 1.0
    lhsT[8, 128:256] = 1.0

    in_maps = []
    tile_of = np.zeros((N_CORES, NT_CORE), np.int64)
    for core in range(N_CORES):
        rhs = np.zeros((KQ, gcap), np.float32)
        rhs[7, :] = NEG  # default color-bias: exp -> 0 at spacers/pads
        col = 0
        for p in range(NT_CORE):
            j = p // NSEG
            t = order[8 * p + core]
            tile_of[core, p] = t
            idx = tile_idx[t]
            k = len(idx)
            seg = lbs[j] + 1
            base = col + 1   # after spacer
            if k:
                ty, tx = divmod(t, NTX)
                cx = tx * TW + TW / 2.0
                cy = ty * TH + TH / 2.0
                mx = means[idx, 0] - cx
                my = means[idx, 1] - cy
                iag, ibg, icg = ia[idx], ib[idx], ic[idx]
                rhs[0, base : base + k] = _round_fp32r(-0.5 * iag)
                rhs[1, base : base + k] = _round_fp32r(-ibg)
                rhs[2, base : base + k] = _round_fp32r(-0.5 * icg)
                rhs[3, base : base + k] = _round_fp32r(iag * mx + ibg * my)
                rhs[4, base : base + k] = _round_fp32r(ibg * mx + icg * my)
                bias = logopac[idx] - 0.5 * (
                    iag * mx * mx + 2 * ibg * mx * my + icg * my * my
                )
                bh, bl = _split_fp32r(bias)
                bch, bcl = _split_fp32r(bias + lnc[idx])
                rhs[5, base : base + k] = bh
                rhs[6, base : base + k] = bl
                rhs[7, base : base + k] = bch
                rhs[8, base : base + k] = bcl
            col += seg
        in_maps.append({"inp": np.concatenate([lhsT, rhs], axis=1)})
    return in_maps, tile_of, gcap, lbs


def _assemble(results, tile_of):
    img = np.zeros((H, W), np.float32)
    for core in range(N_CORES):
        out = np.asarray(results[core]["out"], np.float32)  # [128, NT_CORE]
        for p in range(NT_CORE):
            t = tile_of[core, p]
            ty, tx = divmod(t, NTX)
            img[ty * TH : (ty + 1) * TH, tx * TW : (tx + 1) * TW] = out[
                :, p
            ].reshape(TH, TW)
    return img[None, None].astype(np.float32)


def kernel(means, quats, scales, rgbs, opacities):
    global LAST_RESULT
    from concourse.bass_utils import run_bass_kernel_spmd

    in_maps, tile_of, gcap, lbs = _prep_inputs(means, quats, scales, rgbs, opacities)
    nc = _get_nc(gcap, lbs)
    res = run_bass_kernel_spmd(nc, in_maps, list(range(N_CORES)))
    LAST_RESULT = res
    return _assemble(res.results, tile_of)
